# revision 5
# baseline (speedup 1.0000x reference)
import numpy as np
import jax
import jax.numpy as jnp
from functools import partial

# nn_APDNC: DNC forward pass, data-parallel over batch across 8 NeuronCores.
# Shapes (hardcoded): bs=32, T=32, xd=128, hdim=512, L=2, v_t=2048, W=64, R=4,
# N=1024, E_t=463. Each core gets bs_local=4; weights replicated.

EPS = 1e-8
BS, T, XD = 32, 32, 128
HDIM, L, VT, W, R, N = 512, 2, 2048, 64, 4, 1024
ET = W * R + 3 * W + 3 * R + 3  # 463
NCORES = 8
KTOP = 64  # bottom-K usage slots that can have nonzero allocation weight


# minimax-ish poly for log1p(e), e in [0,1]; max err 3.4e-8. Avoids the
# exp->log fused chain that ICEs the neuron ACT-lowering pass.
_LOG1P_C = [-0.006151470962140602, 0.03484971247929511, -0.09325203898589371,
            0.1658227526890956, -0.2398261605025459, 0.33154861652033457,
            -0.4998385618342527, 0.9999942724811794, 3.3869652969979495e-08]


def _softplus_neg(x):
    """softplus(-x) = -log_sigmoid(x), ICE-safe formulation."""
    e = jnp.exp(-jnp.abs(x))
    p = _LOG1P_C[0]
    for c in _LOG1P_C[1:]:
        p = p * e + c
    return p + jax.nn.relu(-x)


def _lstm_cell(xx, h, c, Wih, Whh, bih, bhh):
    g = xx @ Wih.T + bih + h @ Whh.T + bhh
    i, f, gg, o = jnp.split(g, 4, axis=-1)
    i = jax.nn.sigmoid(i)
    f = jax.nn.sigmoid(f)
    o = jax.nn.sigmoid(o)
    c2 = f * c + i * jnp.tanh(gg)
    return o * jnp.tanh(c2), c2


def _layernorm(t):
    mu = t.mean(-1, keepdims=True)
    var = t.var(-1, keepdims=True)
    return (t - mu) / jnp.sqrt(var + 1e-5)


def _allocation(u_new):
    """Sort-free allocation weighting.

    Reference: sort u ascending (stable), a_sorted = (1-su)*exclusive_cumprod(su),
    unsort. Only the KTOP smallest-usage slots can carry non-negligible weight:
    sum(u) <= T = 32, so prod of the 64 smallest usages <= (32/64)^64 ~ 5e-20,
    which underflows to 0 in f32 exactly as the reference's cumprod does.
    top_k with a tiny index tiebreak reproduces the stable ordering.
    """
    bs = u_new.shape[0]
    iota = jnp.arange(N, dtype=jnp.float32)
    # ascending u == descending -u; tiny iota term breaks ties by lower index
    neg_key = -u_new - iota * 1e-30
    vals, idx = jax.lax.top_k(neg_key, KTOP)  # vals descending
    su = jnp.take_along_axis(u_new, idx, axis=1)  # K smallest u, ascending
    cp = jnp.cumprod(su, axis=1)
    cp = jnp.concatenate([jnp.ones((bs, 1), su.dtype), cp[:, :-1]], axis=1)
    a_k = (1.0 - su) * cp  # (bs, K)
    # scatter back via one-hot matmul (scatter is not supported on trn2)
    onehot = (idx[:, :, None] == iota[None, None, :]).astype(jnp.float32)
    return jnp.einsum("bk,bkn->bn", a_k, onehot)


def _dnc_core(bn_x, memory0, Wy, WE, Wr, Wih0, Whh0, bih0, bhh0, Wih1, Whh1,
              bih1, bhh1):
    """Runs the scan for a batch shard. bn_x: (T, bsl, xd) already batch-normed."""
    bsl = memory0.shape[0]
    lstm_params = [(Wih0, Whh0, bih0, bhh0), (Wih1, Whh1, bih1, bhh1)]

    def step(carry, x_t):
        M, lrw, u, lww, lrv, h, c = carry
        layer_in = jnp.concatenate([x_t, lrv.reshape(bsl, W * R)], axis=1)
        hs, cs = [], []
        for l in range(L):
            Wih, Whh, bih, bhh = lstm_params[l]
            hf, cf = _lstm_cell(layer_in, h[2 * l], c[2 * l], Wih[0], Whh[0],
                                bih[0], bhh[0])
            hb, cb = _lstm_cell(layer_in, h[2 * l + 1], c[2 * l + 1], Wih[1],
                                Whh[1], bih[1], bhh[1])
            layer_in = jnp.concatenate([hf, hb], axis=1)
            hs += [hf, hb]
            cs += [cf, cb]
        h_new = jnp.stack(hs)
        c_new = jnp.stack(cs)
        flat = jnp.transpose(h_new, (1, 0, 2)).reshape(bsl, 2 * L * HDIM)
        vt = flat @ Wy
        iv = _layernorm(flat @ WE)
        i0 = W * R
        rkeys = iv[:, :i0].reshape(bsl, W, R)
        sp_r = _softplus_neg(iv[:, i0:i0 + R]); i0 += R  # rbeta = 1 + sp_r
        wkey = iv[:, i0:i0 + W]; i0 += W
        sp_w = _softplus_neg(iv[:, i0:i0 + 1]); i0 += 1  # wbeta = 1 + sp_w
        erase = jax.nn.sigmoid(iv[:, i0:i0 + W]); i0 += W
        wvec = iv[:, i0:i0 + W]; i0 += W
        fgates = jax.nn.sigmoid(iv[:, i0:i0 + R]); i0 += R
        ag = jax.nn.sigmoid(iv[:, i0:i0 + 1]); i0 += 1
        wg = jax.nn.sigmoid(iv[:, i0:i0 + 1]); i0 += 1

        retention = jnp.prod(1.0 - fgates[:, None, :] * lrw, axis=2)
        u_new = (u + lww - u * lww) * retention
        alloc = _allocation(u_new)

        Mn = M / (jnp.linalg.norm(M, axis=2, keepdims=True) + EPS)
        kn = wkey / (jnp.linalg.norm(wkey, axis=1, keepdims=True) + EPS)
        csim = jnp.einsum("bnw,bw->bn", Mn, kn)
        cw = jax.nn.softmax(csim + sp_w * csim, axis=1)  # wbeta*sim distributed
        ww = wg * (ag * alloc + (1.0 - ag) * cw)
        M_new = (M * (1.0 - ww[:, :, None] * erase[:, None, :])
                 + ww[:, :, None] * wvec[:, None, :])
        Mn2 = M_new / (jnp.linalg.norm(M_new, axis=2, keepdims=True) + EPS)
        rkn = rkeys / (jnp.linalg.norm(rkeys, axis=1, keepdims=True) + EPS)
        sim = jnp.einsum("bnw,bwr->bnr", Mn2, rkn)
        rw = jax.nn.softmax(sim + sp_r[:, None, :] * sim, axis=1)
        rv = jnp.einsum("bnw,bnr->bwr", M_new, rw)
        yt = vt + rv.reshape(bsl, W * R) @ Wr
        return (M_new, rw, u_new, ww, rv, h_new, c_new), yt

    z = lambda *s: jnp.zeros(s, jnp.float32)
    init = (memory0, z(bsl, N, R), z(bsl, N), z(bsl, N), z(bsl, W, R),
            z(2 * L, bsl, HDIM), z(2 * L, bsl, HDIM))
    _, yts = jax.lax.scan(step, init, bn_x)
    return jnp.max(yts, axis=0)


_pmapped = None


def _get_pmapped():
    global _pmapped
    if _pmapped is None:
        _pmapped = jax.pmap(
            _dnc_core,
            in_axes=(0, 0) + (None,) * 11,
            devices=jax.devices()[:NCORES],
        )
    return _pmapped


def kernel(x, memory0, Wy, WE, Wr, Wih0, Whh0, bih0, bhh0, Wih1, Whh1, bih1,
           bhh1):
    # BatchNorm1d (training-mode batch stats over the FULL batch) depends only
    # on x, so compute it before sharding; the rest of the scan is
    # batch-independent.
    x = np.asarray(x, np.float32)
    mu = x.mean(0, keepdims=True)          # (1, T, xd)
    var = x.var(0, keepdims=True)
    bn = (x - mu) / np.sqrt(var + 1e-5)    # (bs, T, xd)
    # (T, bs, xd) -> shard batch -> (ncores, T, bsl, xd)
    bn_t = np.transpose(bn, (1, 0, 2))
    bsl = BS // NCORES
    bn_sh = np.stack([bn_t[:, i * bsl:(i + 1) * bsl] for i in range(NCORES)])
    m0_sh = np.stack([memory0[i * bsl:(i + 1) * bsl] for i in range(NCORES)])

    out_sh = _get_pmapped()(
        jnp.asarray(bn_sh), jnp.asarray(m0_sh), Wy, WE, Wr, Wih0, Whh0, bih0,
        bhh0, Wih1, Whh1, bih1, bhh1)
    out = np.asarray(out_sh).reshape(BS, VT)
    return out.astype(np.float32)
